# revision 5
# baseline (speedup 1.0000x reference)
"""TRN2 Bass kernel for nn_Attention_17935783428543.

Reference computation (per batch b of 4):
  qkv = w_qkv @ X        (X = x[b] as [C=128, N=4096])
  per head h (4 heads, d=32): sim = (q_h * scale)^T k_h ; P = softmax(sim)
  y_h = P @ v_h^T ; out = w_out @ concat_h(y_h^T) + b_out

Sharding: 8 cores = 4 batches x 2 query-halves. Each core computes the full
attention for its batch restricted to 2048 query pixels (all 4096 keys), all
4 heads, including QKV projection and the output projection. No collectives.

Per-core design (v2, "flipped P@V"). The PE streams 1 moving column/cycle
at 2.4GHz; the baseline streamed ~668k columns (sim 262k + P@V 262k +
sums 131k + proj 12k). This version collapses P@V+sums to ~68k columns by
transposing the P@V matmul: per (j-chunk J, head h, i-block b) one matmul
with lhsT = pT[j=128, i=128] (stationary, reloaded each time) and
rhs = [v_h^T | ones] [128 j, 33] accumulates y[i, d] AND the softmax
denominator (33rd column) into PSUM [i=128, 132]. Micro-benchmarks show
such back-to-back small-N matmuls cost ~82 cycles (weight-load-bound), and
that mixing PE tile configs costs ~730 cycles per transition - so the
kernel runs PHASE-SEPARATED per 512-query i-chunk:
  phase 1: 128 sim slots (K=32, N=512, head-rotated across PE row bands to
           keep stationary loads off the critical path), each exp'd from
           the 4-deep PSUM ring into per-slot SBUF pT tiles by a weighted
           3-way engine split (ScalarE exact exp; VectorE and GpSimd
           Schraudolph bit-trick exp).
  phase 2: 512 flip matmuls back-to-back (~82c each), then epilogue:
           per-partition 1/denominator scaling (denominators land on the
           i-partition axis), PE transpose of y[i,hd] -> y^T[hd,i], output
           projection, bias, DMA out.
Predicted ~454k PE cycles/core ~= 190us vs 278us baseline.
"""

import numpy as np
import ml_dtypes

import concourse.mybir as mybir
import concourse.tile as tile
from concourse import bacc
from concourse.bass_utils import run_bass_kernel_spmd
from concourse.masks import make_identity

F32 = mybir.dt.float32
BF16 = mybir.dt.bfloat16
I16 = mybir.dt.int16
NPBF16 = ml_dtypes.bfloat16

B = 4
C = 128
HEADS = 4
D = 32
N = 4096          # pixels per batch (64*64)
NQ = 2048         # query pixels per core
SCALE = D ** -0.5
I_CHUNK = 512
N_I = NQ // I_CHUNK     # 4
N_J = N // 128          # 32

# Schraudolph exp-as-bf16-bits constants: bits = sim*A2 + B2, bitcast bf16
LOG2E = 1.4426950408889634
C_ADJ = 0.0579
A2 = SCALE * 128.0 * LOG2E
B2 = 16256.0 - 128.0 * C_ADJ

# exp engine weights (ScalarE exact, VectorE Schraudolph).
# GPSIMD cannot read PSUM, so it cannot join the exp stream.
W_ACT = 0.56
W_DVE = 0.44

_NC_CACHE = {}


def _build_nc():
    nc = bacc.Bacc("TRN2", target_bir_lowering=False, debug=False, num_devices=8)

    x = nc.dram_tensor("x", [N // 512, C, 512], BF16, kind="ExternalInput").ap()
    wq = nc.dram_tensor("wq", [C, C], BF16, kind="ExternalInput").ap()
    wk = nc.dram_tensor("wk", [C, C], BF16, kind="ExternalInput").ap()
    wv = nc.dram_tensor("wv", [C, C], BF16, kind="ExternalInput").ap()
    wo = nc.dram_tensor("wo", [C, C], BF16, kind="ExternalInput").ap()
    bo = nc.dram_tensor("bo", [C, 1], F32, kind="ExternalInput").ap()
    out = nc.dram_tensor("out", [C, NQ], F32, kind="ExternalOutput").ap()

    with tile.TileContext(nc) as tc:
        with (
            tc.tile_pool(name="const", bufs=1) as cpool,
            tc.tile_pool(name="acts", bufs=1) as apool,
            tc.tile_pool(name="pt", bufs=1) as ptpool,
            tc.tile_pool(name="epi", bufs=2) as epool,
            tc.tile_pool(name="psum_ring", bufs=4, space="PSUM") as pring,
            tc.tile_pool(name="psum_acc", bufs=1, space="PSUM") as pacc,
        ):
            # ---- constants / weights ----
            wq_sb = cpool.tile([C, C], BF16, tag="wq")
            nc.sync.dma_start(wq_sb[:], wq)
            wk_sb = cpool.tile([C, C], BF16, tag="wk")
            nc.sync.dma_start(wk_sb[:], wk)
            wv_sb = cpool.tile([C, C], BF16, tag="wv")
            nc.sync.dma_start(wv_sb[:], wv)
            wo_sb = cpool.tile([C, C], BF16, tag="wo")
            nc.sync.dma_start(wo_sb[:], wo)
            bo_sb = cpool.tile([C, 1], F32, tag="bo")
            nc.sync.dma_start(bo_sb[:], bo)
            ident = cpool.tile([C, C], BF16, tag="ident")
            make_identity(nc, ident[:])

            # warm the ACT exp table during the DMA prologue
            warm = cpool.tile([1, 1], F32, tag="warm")
            nc.vector.memset(warm[:], 0.0)
            nc.scalar.activation(warm[:], warm[:], mybir.ActivationFunctionType.Exp)

            # ---- x DMA (8 chunks so early projections can start early) ----
            x_sb = apool.tile([C, N], BF16, tag="x")
            for g in range(N // 512):
                nc.gpsimd.dma_start(x_sb[:, 512 * g: 512 * (g + 1)], x[g])

            q_all = apool.tile([C, NQ], BF16, tag="q")    # [4h*32c', i]
            k_all = apool.tile([C, N], BF16, tag="k")     # [4h*32c', j]
            # flip-P@V rhs: per (J, h): [v_h^T (32 cols) | ones]
            vTa = apool.tile([C, N_J, HEADS, 33], BF16, tag="vTa")
            nc.vector.memset(vTa[:, :, :, 32], 1.0)

            # round-robin copy engines for projection epilogues
            cp_state = [0]

            def copy_any(dst, src):
                e = cp_state[0] % 2
                cp_state[0] += 1
                if e == 0:
                    nc.scalar.activation(dst, src, mybir.ActivationFunctionType.Copy)
                else:
                    nc.vector.tensor_copy(dst, src)

            # ---- projections (all through the psum ring, N>=128 streams) ----
            def emit_kq(kind, g):
                ps = pring.tile([128, 512], F32, tag="ring", name="proj_ps")
                w_sb = wk_sb if kind == "k" else wq_sb
                dst = k_all if kind == "k" else q_all
                nc.tensor.matmul(
                    ps[:], lhsT=w_sb[:], rhs=x_sb[:, 512 * g: 512 * (g + 1)],
                    start=True, stop=True,
                )
                copy_any(dst[:, 512 * g: 512 * (g + 1)], ps[:])

            def emit_v(g):
                # 4 j-chunks J=4g..4g+3: out[j, hd] = x_J^T wv
                ps = pring.tile([128, 512], F32, tag="ring", name="v_ps")
                for c4 in range(4):
                    Jc = 4 * g + c4
                    nc.tensor.matmul(
                        ps[:, 128 * c4: 128 * (c4 + 1)],
                        lhsT=x_sb[:, 128 * Jc: 128 * (Jc + 1)],
                        rhs=wv_sb[:],
                        start=True, stop=True,
                    )
                for c4 in range(4):
                    Jc = 4 * g + c4
                    for h in range(HEADS):
                        copy_any(
                            vTa[:, Jc, h, 0:32],
                            ps[:, 128 * c4 + 32 * h: 128 * c4 + 32 * h + 32],
                        )

            # k/q for chunk 0 first, then the rest; v last (needed only by
            # the first flip phase, which starts after 128 sim slots)
            emit_kq("k", 0)
            emit_kq("q", 0)
            for g in range(1, 8):
                emit_kq("k", g)
            for g in range(8):
                emit_v(g)
            for g in range(1, 4):
                emit_kq("q", g)

            # ---- 3-way exp engine rotation ----
            exp_acc = [0.0, 0.0]
            exp_w = [W_ACT, W_DVE]

            def emit_exp(pTs, ring):
                for e in range(2):
                    exp_acc[e] += exp_w[e]
                e = max(range(2), key=lambda i: exp_acc[i])
                exp_acc[e] -= 1.0
                if e == 0:
                    nc.scalar.activation(
                        pTs[:], ring[:], mybir.ActivationFunctionType.Exp,
                        scale=SCALE,
                    )
                else:
                    nc.vector.tensor_scalar(
                        pTs[:].bitcast(I16), ring[:], A2, B2,
                        mybir.AluOpType.mult, mybir.AluOpType.add,
                    )

            # ---- attention, phase-separated per i-chunk ----
            for I in range(N_I):
                isl = slice(I_CHUNK * I, I_CHUNK * (I + 1))

                # phase 1: 128 sim slots (h fastest: rotates PE row bands)
                pT_tiles = []
                for s in range(N_J * HEADS):
                    J, h = s // 4, s % 4
                    ring = pring.tile([128, 512], F32, tag="ring", name="simw")
                    nc.tensor.matmul(
                        ring[:],
                        lhsT=k_all[32 * h: 32 * h + 32, 128 * J: 128 * (J + 1)],
                        rhs=q_all[32 * h: 32 * h + 32, isl],
                        start=True, stop=True,
                        tile_position=(32 * h, 0),
                    )
                    pTs = ptpool.tile([128, 512], BF16, tag=f"pT{s}", name=f"pT{s}")
                    emit_exp(pTs, ring)
                    pT_tiles.append(pTs)

                # phase 2: flip P@V + fused denominators, back-to-back
                # (PSUM tiles are bank-granular: pack 2 i-blocks per bank)
                y_ps = [
                    pacc.tile([128, 2, HEADS, 33], F32, tag=f"y{t}", name=f"y{t}",
                              padded_shape=[128, 2, HEADS, 64])
                    for t in range(2)
                ]
                for b in range(4):
                    for h in range(HEADS):
                        for J in range(N_J):
                            s = 4 * J + h
                            nc.tensor.matmul(
                                y_ps[b // 2][:, b % 2, h, :],
                                lhsT=pT_tiles[s][:, 128 * b: 128 * (b + 1)],
                                rhs=vTa[:, J, h, :],
                                start=(J == 0), stop=(J == N_J - 1),
                            )

                # epilogue: normalize (denominators on partition axis),
                # transpose y[i,hd] -> y^T[hd,i], project, bias, store
                ynorm = epool.tile([128, 512], BF16, tag="ynorm")
                for b in range(4):
                    r4 = epool.tile([128, HEADS], F32, tag=f"r4_{b}", name=f"r4_{b}")
                    nc.vector.reciprocal_approx_fast(
                        r4[:], y_ps[b // 2][:, b % 2, :, 32]
                    )
                    for h in range(HEADS):
                        nc.vector.tensor_scalar_mul(
                            ynorm[:, 128 * b + 32 * h: 128 * b + 32 * h + 32],
                            y_ps[b // 2][:, b % 2, h, 0:32],
                            r4[:, h: h + 1],
                        )
                yT_ps = pacc.tile([128, 4, 128], BF16, tag="yT",
                                  padded_shape=[128, 4, 256])
                for b in range(4):
                    nc.tensor.transpose(
                        yT_ps[:, b, :], ynorm[:, 128 * b: 128 * (b + 1)], ident[:]
                    )
                yT_sb = epool.tile([128, 4, 128], BF16, tag="yT_sb")
                nc.scalar.activation(
                    yT_sb[:], yT_ps[:], mybir.ActivationFunctionType.Copy
                )
                op = pacc.tile([128, 512], F32, tag="op")
                nc.tensor.matmul(
                    op[:], lhsT=wo_sb[:], rhs=yT_sb[:, :, :],
                    start=True, stop=True,
                )
                out_sb = epool.tile([128, 512], F32, tag="out_sb")
                nc.vector.tensor_scalar_add(out_sb[:], op[:], bo_sb[:, :])
                nc.sync.dma_start(out[:, isl], out_sb[:])

    nc.compile()
    return nc


def kernel(x, w_qkv, w_out, b_out, _trace=False):
    if "nc" not in _NC_CACHE:
        _NC_CACHE["nc"] = _build_nc()
    nc = _NC_CACHE["nc"]

    x = np.asarray(x, dtype=np.float32).reshape(B, C, N)
    w_qkv = np.asarray(w_qkv, dtype=np.float32)
    w_out = np.asarray(w_out, dtype=np.float32)
    b_out = np.asarray(b_out, dtype=np.float32)

    wq = np.ascontiguousarray(w_qkv[0:C].T).astype(NPBF16)
    wk = np.ascontiguousarray(w_qkv[C: 2 * C].T).astype(NPBF16)
    wv = np.ascontiguousarray(w_qkv[2 * C: 3 * C].T).astype(NPBF16)
    wo = np.ascontiguousarray(w_out.T).astype(NPBF16)
    bo = np.ascontiguousarray(b_out.reshape(C, 1))

    in_maps = []
    for core in range(8):
        b, half = core >> 1, core & 1
        xb = x[b]
        if half:
            xb = np.concatenate([xb[:, NQ:], xb[:, :NQ]], axis=1)
        xb_c = np.ascontiguousarray(
            xb.reshape(C, N // 512, 512).transpose(1, 0, 2)
        ).astype(NPBF16)
        in_maps.append(
            {
                "x": xb_c,
                "wq": wq,
                "wk": wk,
                "wv": wv,
                "wo": wo,
                "bo": bo,
            }
        )

    res = run_bass_kernel_spmd(nc, in_maps, list(range(8)), trace=_trace)

    full = np.empty((B, C, N), np.float32)
    for core in range(8):
        b, half = core >> 1, core & 1
        full[b][:, NQ * half: NQ * (half + 1)] = res.results[core]["out"]
    out = full.reshape(B, C, 64, 64)
    if _trace:
        return out, res
    return out
